# revision 66
# baseline (speedup 1.0000x reference)
"""NNConv (gnn_message_passing) Bass kernel for 8 Trainium2 NeuronCores.

Strategy (edge-parallel, dst-sharded):
- Host relabels nodes with a permutation so that the 16384 nodes form 128
  "windows" of 128 nodes, each window receiving exactly U edges (by
  destination).  Core c owns windows [16c, 16c+16): 2048 nodes / 8192 edges.
  Pure SPMD; per-core variation lives only in the input data.
- Per 128-edge tile, on device (all matmuls bf16):
    P   = attr_aug^T @ Aaug            (PE matmul -> PSUM f32, K=3)
    q   = relu(P) * x[src] broadcast   (one DVE scalar_tensor_tensor per
                                        1024-col unit, bf16 out)
    agg += onehot(dst)^T @ q           (PE matmul; zero-step output AP sums
                                        the c_in axis while contracting edges)
  The scatter matmuls for tile t are emitted during tile t+1's generation so
  the PE never stalls waiting for the DVE.  The root term (x @ root) and the
  node bias ride in a single augmented matmul that opens each window's PSUM
  accumulation.
- h1 is exchanged between layers with an AllGather (bf16).
"""

import numpy as np
import ml_dtypes
from contextlib import ExitStack

import concourse.bass as bass
import concourse.tile as tile
from concourse import bacc, mybir
from concourse.bass import IndirectOffsetOnAxis
from concourse.bass_utils import run_bass_kernel_spmd

dt = mybir.dt
BF16 = ml_dtypes.bfloat16

N = 16384
E = 65536
NCORES = 8
P = 128                 # partitions / edges per tile
WINDOWS = 128           # global 128-node windows
WPC = WINDOWS // NCORES  # 16 windows per core
NPC = N // NCORES        # 2048 nodes per core
COUT = 64
CIN1 = 8
CIN2 = 64
# AllGather chunk boundaries in windows-per-core: early chunks overlap
# layer-1 compute, small tail chunks shrink the layer-boundary wait
AGB = [0, 4, 8, 12, 16]

_cached = {}


def _build_program(U):
    """Build the SPMD Bass program. U = edges per window (multiple of 128)."""
    T = U // P    # tiles per window
    NT = WPC * T  # tiles per core per layer
    EPC = WPC * U

    nc = bacc.Bacc("TRN2", target_bir_lowering=False, debug=False,
                   num_devices=NCORES)

    # attr and A replicated at partition quadrants 0/32/64/96 so four K=3
    # generator matmuls can run concurrently in distinct PE row groups.
    attrT_d = nc.dram_tensor("attrT", [99, EPC], dt.bfloat16, kind="ExternalInput").ap()
    srcw_d = nc.dram_tensor("srcw", [P, NT], dt.int32, kind="ExternalInput").ap()
    A1_d = nc.dram_tensor("A1aug", [99, CIN1 * COUT], dt.bfloat16, kind="ExternalInput").ap()
    A2_d = nc.dram_tensor("A2aug", [99, CIN2 * COUT], dt.bfloat16, kind="ExternalInput").ap()
    # host-precomputed layer-1 gathered features and per-tile one-hot
    # matrices (bf16 for L1 scatter, fp8 for the L2 DoubleRow scatter)
    xsg_d = nc.dram_tensor("xsg", [P, NT * CIN1], dt.bfloat16, kind="ExternalInput").ap()
    oh1_d = nc.dram_tensor("oh1", [P, NT * P], dt.bfloat16, kind="ExternalInput").ap()
    oh2_d = nc.dram_tensor("oh2", [P, NT * P], dt.float8e4, kind="ExternalInput").ap()
    xT_d = nc.dram_tensor("xT9", [CIN1 + 1, NPC], dt.bfloat16, kind="ExternalInput").ap()
    r1_d = nc.dram_tensor("r1aug", [CIN1 + 1, COUT], dt.bfloat16, kind="ExternalInput").ap()
    r2_d = nc.dram_tensor("r2aug", [CIN2 + 1, COUT], dt.bfloat16, kind="ExternalInput").ap()
    iota_d = nc.dram_tensor("iota", [P, P], dt.bfloat16, kind="ExternalInput").ap()
    out_d = nc.dram_tensor("out", [NPC, COUT], dt.float32, kind="ExternalOutput").ap()

    with tile.TileContext(nc) as tc, ExitStack() as ctx:
        consts = ctx.enter_context(tc.tile_pool(name="consts", bufs=1))
        xgp = ctx.enter_context(tc.tile_pool(name="xgp", bufs=6))
        ohp = ctx.enter_context(tc.tile_pool(name="ohp", bufs=6))
        qp = ctx.enter_context(tc.tile_pool(name="qp", bufs=10))
        trp = ctx.enter_context(tc.tile_pool(name="trp", bufs=4))
        outp = ctx.enter_context(tc.tile_pool(name="outp", bufs=3))
        pp = ctx.enter_context(tc.tile_pool(name="pp", bufs=3, space="PSUM"))
        aggp = ctx.enter_context(tc.tile_pool(name="aggp", bufs=2, space="PSUM"))
        dramp = ctx.enter_context(tc.tile_pool(name="dram", bufs=1, space="DRAM"))

        A1_s = consts.tile([99, CIN1 * COUT], dt.bfloat16)
        nc.sync.dma_start(A1_s[:], A1_d[:])
        A2_s = consts.tile([99, CIN2 * COUT], dt.bfloat16)
        nc.sync.dma_start(A2_s[:], A2_d[:])
        # whole-layer metadata loaded once (shared by both layers); per-window
        # dma_starts cost ~1us of SWDGE queue latency each and serialize L1
        srcw_s = consts.tile([P, NT], dt.int32)
        nc.sync.dma_start(srcw_s[:], srcw_d[:])
        xT9_s = consts.tile([CIN1 + 1, NPC], dt.bfloat16)
        nc.sync.dma_start(xT9_s[:], xT_d[:])
        # big constants are loaded in 8 column-chunks on the Activation
        # engine's HWDGE queue, keeping the Sync queue free for the
        # layer-critical hloc/out writes; L1-needed tensors stream first
        attr_s = consts.tile([99, EPC], dt.bfloat16)
        xsg_s = consts.tile([P, NT * CIN1], dt.bfloat16)
        oh1_s = consts.tile([P, NT * P], dt.bfloat16)
        oh2_s = consts.tile([P, NT * P], dt.float8e4)
        for big_s, big_d in ((xsg_s, xsg_d), (oh1_s, oh1_d),
                             (attr_s, attrT_d), (oh2_s, oh2_d)):
            cw = big_s.shape[1] // 8
            for j in range(8):
                nc.scalar.dma_start(big_s[:, j * cw:(j + 1) * cw],
                                    big_d[:, j * cw:(j + 1) * cw])
        iota_s = consts.tile([P, P], dt.bfloat16)
        nc.sync.dma_start(iota_s[:], iota_d[:])
        r1_s = consts.tile([CIN1 + 1, COUT], dt.bfloat16)
        nc.sync.dma_start(r1_s[:], r1_d[:])
        r2_s = consts.tile([CIN2 + 1, COUT], dt.bfloat16)
        nc.sync.dma_start(r2_s[:], r2_d[:])
        # ping-pong lhsT buffers for the layer-2 root matmul: rows 0:64 get
        # h1^T via transpose-DMA each window, row 64 stays all-ones.
        h1T = [consts.tile([CIN2 + 1, P], dt.bfloat16, name=f"h1T{i}",
                           tag=f"h1T{i}")
               for i in range(2)]
        for hT in h1T:
            nc.vector.tensor_scalar(
                out=hT[CIN2:CIN2 + 1, :], in0=iota_s[0:1, :], scalar1=-1.0,
                scalar2=None, op0=mybir.AluOpType.is_ge)

        # h1 slice (local) and allgathered h1 (global), bf16
        hloc = dramp.tile([NPC, COUT], dt.bfloat16)
        hglob = dramp.tile([N, COUT], dt.bfloat16)

        def layer(is_l1, after_window=None):
            cin = CIN1 if is_l1 else CIN2
            cols = cin * COUT           # 512 or 4096
            A_s = A1_s if is_l1 else A2_s
            gather_src = hglob[:]
            nunits = max(1, cols // 1024)
            ucols = min(cols, 1024)
            uich = ucols // COUT        # 8 (L1) or 16 (L2)

            aggws = {}
            state_pair = [None]
            pend_q = []
            pending = None  # (oh, scats, w, is_last_tile_of_window)

            def emit_scat(p):
                oh, scats, w, last = p
                aggw = aggws[w]
                nmm = sum(qt.shape[1] // 512 if kind in ("fat", "fatT")
                          else (qt.shape[1] // (2 * COUT)) // 8 if kind == "dr"
                          else 1
                          for kind, qt in scats)
                k = 0
                for kind, qt in scats:
                    if kind == "fat":
                        # qt holds (i,o)-ordered data
                        for h in range(qt.shape[1] // 512):
                            k += 1
                            hi = 512 // COUT
                            q3 = qt[:, h * 512:(h + 1) * 512].rearrange(
                                "p (i o) -> p i o", i=hi)
                            nc.tensor.matmul(
                                aggw[:].unsqueeze(1).broadcast_to([P, hi, COUT]),
                                lhsT=oh[:], rhs=q3,
                                start=False, stop=(last and k == nmm),
                                skip_group_check=True)
                    elif kind == "fatT":
                        # qt holds (o,i)-ordered data; stream it (i-outer,
                        # o-inner) so the zero-stride PSUM revisit is 64
                        # columns apart (same-address back-to-back writes
                        # lose accumulations).
                        ich = qt.shape[1] // COUT
                        q3 = qt[:].rearrange("p (o i) -> p i o", i=ich)
                        nh = max(1, (COUT * ich) // 512)
                        hi = ich // nh
                        for h in range(nh):
                            k += 1
                            nc.tensor.matmul(
                                aggw[:].unsqueeze(1).broadcast_to(
                                    [P, hi, COUT]),
                                lhsT=oh[:],
                                rhs=q3[:, h * hi:(h + 1) * hi, :],
                                start=False, stop=(last and k == nmm),
                                skip_group_check=True)
                    elif kind == "dr":
                        # qt is a tile-pair unit [P, 2048] fp8e4 ((c,i,o));
                        # oh is the paired one-hot [P, 256] fp8e4.  DoubleRow
                        # contracts both tiles' 128 edges in one pass.
                        ich = qt.shape[1] // (2 * COUT)
                        q4 = qt[:].rearrange("p (c i o) -> p c i o", c=2,
                                             i=ich)
                        oh3 = oh[:].rearrange("p (c v) -> p c v", c=2)
                        for h in range(ich // 8):
                            k += 1
                            nc.tensor.matmul(
                                aggw[:].unsqueeze(1).broadcast_to(
                                    [P, 8, COUT]),
                                lhsT=oh3, rhs=q4[:, :, h * 8:(h + 1) * 8, :],
                                start=False, stop=(last and k == nmm),
                                skip_group_check=True,
                                perf_mode=mybir.MatmulPerfMode.DoubleRow)
                    else:  # thin: qt is msg [P, COUT]
                        k += 1
                        nc.tensor.matmul(
                            aggw[:], lhsT=oh[:], rhs=qt[:],
                            start=False, stop=(last and k == nmm),
                            skip_group_check=True)
                if last:
                    # finalize window on ScalarE: copy PSUM -> SBUF, write out
                    if is_l1:
                        hw_ = outp.tile([P, COUT], dt.bfloat16, tag="h1w")
                        nc.scalar.copy(hw_[:], aggw[:])
                        nc.sync.dma_start(hloc[w * P:(w + 1) * P, :], hw_[:])
                    else:
                        ow = outp.tile([P, COUT], dt.float32, tag="outw")
                        nc.scalar.copy(ow[:], aggw[:])
                        nc.sync.dma_start(out_d[w * P:(w + 1) * P, :], ow[:])
                    if after_window is not None:
                        after_window(w)

            for g in range(NT):
                w, t = divmod(g, T)
                if t == 0:
                    aggw = aggp.tile([P, COUT], dt.float32, tag="aggw")
                    aggws[w] = aggw
                    # root matmul (with bias folded in) opens the accumulation
                    if is_l1:
                        nc.tensor.matmul(aggw[:],
                                         lhsT=xT9_s[:, w * P:(w + 1) * P],
                                         rhs=r1_s[:], start=True, stop=False)
                    else:
                        hT = h1T[w % 2]
                        nc.sync.dma_start_transpose(
                            hT[0:CIN2, :], hloc[w * P:(w + 1) * P, :])
                        nc.tensor.matmul(aggw[:], lhsT=hT[:], rhs=r2_s[:],
                                         start=True, stop=False)

                # per-unit evacuation mode, balancing DVE / ScalarE / PE
                if is_l1:
                    modes = ["stt_fat"]
                    # layer-1 gathered features and one-hots are host data
                    xg = xsg_s[:, g * cin:(g + 1) * cin]
                    oh = oh1_s[:, g * P:(g + 1) * P]
                else:
                    xg = xgp.tile([P, cin], dt.bfloat16, name="xg", tag="xg2")
                    nc.gpsimd.indirect_dma_start(
                        out=xg[:], out_offset=None, in_=gather_src,
                        in_offset=IndirectOffsetOnAxis(ap=srcw_s[:, g:g + 1],
                                                       axis=0))
                    # layer 2 scatters tile PAIRS with fp8 DoubleRow matmuls:
                    # q tiles are written in fp8e4, two tiles per buffer,
                    # halving the PE scatter stream.
                    modes = ["dr", "dr", "dr", "dr"]
                    half = g % 2
                    if half == 0:
                        ohAB = oh2_s[:, g * P:(g + 2) * P]
                        qABs = [qp.tile([P, 2 * 1024], dt.float8e4,
                                        name=f"qAB{u}", tag=f"qAB{u}")
                                for u in range(nunits)]
                        state_pair[0] = (ohAB, qABs)
                    ohAB, qABs = state_pair[0]

                scats = []
                for u in range(nunits):
                    mode = modes[u]
                    pu = pp.tile([P, ucols], dt.float32, name="pu", tag="pu",
                                 padded_shape=[P, 1024])
                    for h in range(ucols // 512):
                        c0 = u * ucols + h * 512
                        # spread the K=3 matmuls over the 4 PE row groups
                        rg = 32 * ((u * (ucols // 512) + h) % 4) if cols >= 2048 \
                            else 32 * (g % 4)
                        nc.tensor.matmul(
                            pu[:, h * 512:(h + 1) * 512],
                            lhsT=attr_s[rg:rg + 3, g * P:(g + 1) * P],
                            rhs=A_s[rg:rg + 3, c0:c0 + 512],
                            start=True, stop=True, tile_position=(rg, 0))
                    xg_sl = xg[:, u * uich:(u + 1) * uich]
                    if mode == "dr":
                        qh = qABs[u][:, half * 1024:(half + 1) * 1024]
                        nc.vector.scalar_tensor_tensor(
                            out=qh.rearrange("p (i o) -> p i o", i=uich),
                            in0=pu[:].rearrange("p (i o) -> p i o", i=uich),
                            scalar=0.0,
                            in1=xg_sl.to_broadcast([P, uich, COUT]),
                            op0=mybir.AluOpType.max, op1=mybir.AluOpType.mult)
                    elif mode == "stt_fat":
                        qt = qp.tile([P, ucols], dt.bfloat16, name="qt",
                                     tag="q1" if is_l1 else "q2")
                        nc.vector.scalar_tensor_tensor(
                            out=qt[:].rearrange("p (i o) -> p i o", i=uich),
                            in0=pu[:].rearrange("p (i o) -> p i o", i=uich),
                            scalar=0.0,
                            in1=xg_sl.to_broadcast([P, uich, COUT]),
                            op0=mybir.AluOpType.max, op1=mybir.AluOpType.mult)
                        scats.append(("fat", qt))
                    else:
                        # ScalarE evacuates relu(P) in (o,i) order; DVE then
                        # multiplies by x[src] at 2x rate (all-bf16, step-1).
                        ru = qp.tile([P, ucols], dt.bfloat16, name="ru",
                                     tag="ru1" if is_l1 else "ru2")
                        nc.scalar.activation(
                            out=ru[:].rearrange("p (o i) -> p o i", i=uich),
                            in_=pu[:].rearrange("p (i o) -> p o i", i=uich),
                            func=mybir.ActivationFunctionType.Relu)
                        qt = qp.tile([P, ucols], dt.bfloat16, name="qtT",
                                     tag="qT1" if is_l1 else "qT2")
                        q3 = qt[:].rearrange("p (o i) -> p o i", i=uich)
                        nc.vector.tensor_tensor(
                            out=q3,
                            in0=ru[:].rearrange("p (o i) -> p o i", i=uich),
                            in1=xg_sl.unsqueeze(1).broadcast_to(
                                [P, COUT, uich]),
                            op=mybir.AluOpType.mult)
                        if mode == "se_fat":
                            scats.append(("fatT", qt))
                        else:  # se_redu: reduce i on DVE, thin scatter
                            msgf = trp.tile([P, COUT], dt.float32,
                                            name="msgf", tag="msgf")
                            nc.vector.tensor_reduce(
                                out=msgf[:], in_=q3,
                                axis=mybir.AxisListType.X,
                                op=mybir.AluOpType.add)
                            msgb = trp.tile([P, COUT], dt.bfloat16,
                                            name="msgb", tag="msgb")
                            nc.vector.tensor_copy(msgb[:], msgf[:])
                            scats.append(("thin", msgb))

                if is_l1:
                    # defer scatters two tiles so the stt latency never
                    # stalls the PE
                    pend_q.append((oh, scats, w, t == T - 1))
                    if len(pend_q) > 2:
                        emit_scat(pend_q.pop(0))
                else:
                    if half == 0 and pending is not None:
                        emit_scat(pending)
                        pending = None
                    if half == 1:
                        pending = (ohAB, [("dr", q) for q in qABs], w,
                                   t == T - 1)
            for p in pend_q:
                emit_scat(p)
            if pending is not None:
                emit_scat(pending)

        # h1 is exchanged in chunks so the early AllGathers overlap layer-1
        # compute and the tail chunks are small.  hglob layout is chunk-major:
        # chunk j holds local rows [AGB[j]*128, AGB[j+1]*128) of every core,
        # cores stacked.  Gather indices are remapped on the host to match.
        def emit_ag(j):
            r0, r1 = AGB[j] * P, AGB[j + 1] * P
            nc.gpsimd.collective_compute(
                "AllGather", mybir.AluOpType.bypass,
                replica_groups=[list(range(NCORES))],
                ins=[hloc[r0:r1, :].opt()],
                outs=[hglob[r0 * NCORES:r1 * NCORES, :].opt()])

        def ag_after_window(w):
            if (w + 1) in AGB:
                emit_ag(AGB.index(w + 1) - 1)

        layer(True, after_window=ag_after_window)
        layer(False)

    nc.compile()
    return nc


def _pack(edge_index):
    """Relabel nodes into 128 windows of 128 nodes / exactly U edges each.

    Returns (perm, U, order) where perm[orig_node] = new node id and
    order = edge permutation grouping edges by destination window, padded.
    """
    dst = np.asarray(edge_index[1], dtype=np.int64)
    deg = np.bincount(dst, minlength=N).astype(np.int64)
    # LPT greedy: descending degree, least-loaded window with free slots
    nodes = np.argsort(-deg, kind="stable")
    loads = np.zeros(WINDOWS, dtype=np.int64)
    slots = np.zeros(WINDOWS, dtype=np.int64)
    wof = np.empty(N, dtype=np.int64)  # window of node
    for v in nodes:
        open_w = np.flatnonzero(slots < P)
        w = open_w[np.argmin(loads[open_w])]
        wof[v] = w
        loads[w] += deg[v]
        slots[w] += 1
    # repair toward exact target load by swapping nodes between windows
    target = E // WINDOWS
    if loads.max() > target:
        by_wd = {}  # (window, degree) -> list of nodes
        for v in range(N):
            by_wd.setdefault((wof[v], deg[v]), []).append(v)
        for _ in range(100000):
            over = int(np.argmax(loads))
            under = int(np.argmin(loads))
            if loads[over] <= target:
                break
            delta = min(loads[over] - target, target - loads[under])
            # find a swap pair with degree difference = d, largest d first
            done = False
            for d in range(int(delta), 0, -1):
                for da in range(int(deg.max()), d - 1, -1):
                    la = by_wd.get((over, da))
                    lb = by_wd.get((under, da - d))
                    if la and lb:
                        a, b = la.pop(), lb.pop()
                        wof[a], wof[b] = under, over
                        by_wd.setdefault((under, da), []).append(a)
                        by_wd.setdefault((over, da - d), []).append(b)
                        loads[over] -= d
                        loads[under] += d
                        done = True
                        break
                if done:
                    break
            if not done:
                break
    U = int(np.ceil(loads.max() / P) * P)
    # perm: nodes sorted by window -> new ids
    new_order = np.argsort(wof * N + np.arange(N), kind="stable")
    perm = np.empty(N, dtype=np.int64)
    perm[new_order] = np.arange(N)
    # edge order: group by destination window, pad each window to U
    ew = wof[dst]
    eorder = np.argsort(ew, kind="stable")
    counts = np.bincount(ew, minlength=WINDOWS)
    padded = np.full(WINDOWS * U, -1, dtype=np.int64)
    pos = 0
    for w in range(WINDOWS):
        c = int(counts[w])
        padded[w * U:w * U + c] = eorder[pos:pos + c]
        pos += c
    return perm, U, padded


def kernel(x, edge_index, edge_attr, A1, b1, A2, b2, root1, bias1, root2, bias2):
    x = np.asarray(x, dtype=np.float32)
    edge_index = np.asarray(edge_index)
    edge_attr = np.asarray(edge_attr, dtype=np.float32)

    perm, U, padded = _pack(edge_index)
    T = U // P
    NT = WPC * T
    key = U
    if key not in _cached:
        _cached[key] = _build_program(U)
    nc = _cached[key]

    src = np.asarray(edge_index[0], dtype=np.int64)
    dst = np.asarray(edge_index[1], dtype=np.int64)
    valid = padded >= 0
    pe = np.where(valid, padded, 0)
    # per padded-edge data
    a01 = edge_attr[pe]                      # [W*U, 2]
    aug = valid.astype(np.float32)
    attr3 = np.stack([a01[:, 0] * aug, a01[:, 1] * aug, aug]).astype(BF16)
    attrT_all = np.zeros((99, attr3.shape[1]), dtype=BF16)
    for rg in range(4):
        attrT_all[32 * rg:32 * rg + 3] = attr3
    # gather-index remap to the chunk-major hglob layout produced by the
    # split AllGather (chunk boundaries AGB, in windows per core)
    nn = np.arange(N, dtype=np.int64)
    cc_ = nn // NPC
    qq = nn % NPC
    agb = np.asarray(AGB, dtype=np.int64)
    jj = np.searchsorted(agb, qq // P, side="right") - 1
    base = agb[jj] * P
    csz = (agb[jj + 1] - agb[jj]) * P
    idxmap = base * NCORES + cc_ * csz + (qq - base)
    srcn_all = np.where(valid, idxmap[perm[src[pe]]], 0).astype(np.int32)
    dstn = perm[dst[pe]]
    wof_e = np.arange(WINDOWS).repeat(U)
    dstrel_all = np.where(valid, dstn - wof_e * P, 0).astype(np.float32)

    x_pi = np.empty_like(x)
    x_pi[perm] = x
    x_bf = x_pi.astype(BF16)
    # xbf rows live at remapped positions so one srcw feeds both layers
    x_bf_remap = np.empty_like(x_bf)
    x_bf_remap[idxmap] = x_bf

    def rep4(Aaug3):
        out = np.zeros((99, Aaug3.shape[1]), dtype=BF16)
        for rg in range(4):
            out[32 * rg:32 * rg + 3] = Aaug3
        return out
    A1aug = rep4(np.concatenate([A1, b1[None, :]], axis=0).astype(BF16))
    A2aug = rep4(np.concatenate([A2, b2[None, :]], axis=0).astype(BF16))
    r1aug = np.concatenate([root1, bias1[None, :]], axis=0).astype(BF16)
    r2aug = np.concatenate([root2, bias2[None, :]], axis=0).astype(BF16)
    iota_np = np.broadcast_to(np.arange(P, dtype=np.float32), (P, P)).astype(BF16)
    FP8 = ml_dtypes.float8_e4m3
    shared = {
        "A1aug": np.asarray(A1aug), "A2aug": np.asarray(A2aug),
        "r1aug": np.asarray(r1aug), "r2aug": np.asarray(r2aug),
        "iota": np.asarray(iota_np),
    }
    EPC = WPC * U
    in_maps = []
    ones9 = np.ones((1, NPC), dtype=BF16)
    vrange = np.arange(P, dtype=np.float32)
    for c in range(NCORES):
        s = c * EPC
        m = dict(shared)
        m["attrT"] = attrT_all[:, s:s + EPC].copy()
        # [P, NT] with column (w*T + t) = edges [wU + t*128 : wU + (t+1)*128)
        srcw = np.ascontiguousarray(srcn_all[s:s + EPC].reshape(NT, P).T)
        m["srcw"] = srcw
        dstw = dstrel_all[s:s + EPC].reshape(NT, P).T  # [P, NT]
        # host-precomputed layer-1 gathered features and one-hot matrices
        m["xsg"] = x_bf_remap[srcw].reshape(P, NT * CIN1)
        oh = (dstw[:, :, None] == vrange[None, None, :])  # [P, NT, 128]
        m["oh1"] = oh.astype(BF16).reshape(P, NT * P)
        m["oh2"] = oh.astype(FP8).reshape(P, NT * P)
        xTc = np.ascontiguousarray(x_bf[c * NPC:(c + 1) * NPC].T)
        m["xT9"] = np.concatenate([xTc, ones9], axis=0)
        in_maps.append(m)

    res = run_bass_kernel_spmd(nc, in_maps, list(range(NCORES)),
                               **kernel.run_kwargs)
    kernel.last_result = res
    out_pi = np.concatenate([res.results[c]["out"] for c in range(NCORES)], axis=0)
    return out_pi[perm]


kernel.run_kwargs = {}
kernel.last_result = None
